# revision 6
# baseline (speedup 1.0000x reference)
"""Trainium2 Bass kernel for nn_DAWNBlock (moe_routing).

Strategy: data-parallel over batch B=8 (one batch row per NeuronCore).

Per core (call 1):
  x [2048, 2048] -> PE-transpose -> x^T -> fp32 projection (col-packed) -> h [64, 2048]
  k=65-augmented fp32r logits (exc bias folded in as an extra contraction row)
  ACT exp -> bf16 E with fp32 per-pool row-sum accumulators (softmax denominators Z)
  bf16 col-packed PE matmuls accumulate importance-weighted pooled dense scores
  outputs: packed dense/colsum, h, per-(s,pool) weights a = imp/Z

Host: candidate top-32 per pool from approx dense; gathers exact emb columns.
Call 2: exact fp32 recompute of candidate logits/exp/pooled-dense on device.
Host: final top-k set + renormalize + scatter; aux loss reduced across cores.
"""
import numpy as np
from contextlib import ExitStack

import concourse.bass as bass
import concourse.mybir as mybir
import concourse.tile as tile
from concourse import bacc
from concourse.bass_utils import run_bass_kernel_spmd
from concourse.masks import make_identity

F32 = mybir.dt.float32
F32R = mybir.dt.float32r
BF16 = mybir.dt.bfloat16
AF = mybir.ActivationFunctionType
OP = mybir.AluOpType

B, S, D, DS = 8, 2048, 2048, 64
NF, NR, NV = 2048, 1024, 1536
NTOT = NF + NR + NV           # 4608 (the trailing K pool of neuron_emb is unused)
KF, KR, KV = 8, 4, 6
NCAND = 32                    # refined candidates per pool
NT = S // 128                 # 16 s-tiles
NCHUNK = NTOT // 512          # 9 n-chunks of 512

# phases: (name, first chunk, num chunks, Z accumulation slot)
# Zall columns: 0=F, 1=R, 2=V ; Zs scratch: 0,1 for F halves; 2,3 for V halves
PHASES = [("F1", 0, 2), ("F2", 2, 2), ("R", 4, 2), ("V1", 6, 2), ("V2", 8, 1)]

_CACHE = {}
LAST_TIMING = {}


def _build_call1():
    nc = bacc.Bacc("TRN2", target_bir_lowering=False, debug=False, num_devices=8)

    xb = nc.dram_tensor("xb", [S, D], F32, kind="ExternalInput")
    imp2d = nc.dram_tensor("imp2d", [128, NT], F32, kind="ExternalInput")
    Wt = nc.dram_tensor("Wt", [D, DS], F32, kind="ExternalInput")
    bproj = nc.dram_tensor("bproj", [DS, 1], F32, kind="ExternalInput")
    embT = nc.dram_tensor("embT", [65, NTOT], F32, kind="ExternalInput")

    dpacked = nc.dram_tensor("dpacked", [128, 3 * 512], F32, kind="ExternalOutput")
    h_out = nc.dram_tensor("h_out", [65, S], F32, kind="ExternalOutput")
    a48_out = nc.dram_tensor("a48_out", [128, 3 * NT], F32, kind="ExternalOutput")

    with tile.TileContext(nc) as tc, ExitStack() as ctx:
        sb = ctx.enter_context(tc.tile_pool(name="sb", bufs=1))
        sb2 = ctx.enter_context(tc.tile_pool(name="sb2", bufs=1))
        ps = ctx.enter_context(tc.tile_pool(name="ps", bufs=1, space="PSUM"))

        # ---- persistent setup ----
        t_W = sb.tile([128, 16, DS], F32, tag="W")
        nc.sync.dma_start(out=t_W, in_=Wt.ap().rearrange("(k p) d -> p k d", p=128))
        t_bp = sb.tile([DS, 1], F32, tag="bp")
        nc.sync.dma_start(out=t_bp, in_=bproj.ap())
        t_imp = sb.tile([128, NT], F32, tag="imp")
        nc.sync.dma_start(out=t_imp, in_=imp2d.ap())
        t_emb = sb.tile([65, NTOT], F32, tag="emb")
        nc.sync.dma_start(out=t_emb, in_=embT.ap())
        r_emb = sb.tile([65, NTOT], F32R, tag="remb")
        nc.vector.tensor_copy(r_emb, t_emb)

        ident = sb.tile([128, 128], F32, tag="ident")
        make_identity(nc, ident)

        h_buf = sb.tile([65, S], F32, tag="h")
        nc.vector.memset(h_buf[64:65, :], 1.0)
        r_h = sb.tile([65, S], F32R, tag="rh")
        ones_row = sb.tile([1, S], F32, tag="ones_row")
        nc.vector.memset(ones_row, 1.0)
        nc.vector.tensor_copy(r_h[64:65, :], ones_row)
        a48 = sb.tile([128, 3 * NT], F32, tag="a48")

        # persistent dense accumulators: chunk c -> bank c//4, col-group c%4
        pd = []
        for i in range(3):
            pd_i = ps.tile([128, 512], F32, tag=f"pd{i}", name=f"pd{i}")
            pd.append(pd_i)

        for g in range(4):          # s-groups of 512
            for pair in range(2):   # pairs of 128-s tiles
                xTp = sb2.tile([128, 16, 256], F32, tag="xT", bufs=2)
                for tl in range(2):
                    tt = 4 * g + 2 * pair + tl
                    x_t = sb2.tile([128, D], F32, tag="xt", bufs=3)
                    nc.sync.dma_start(out=x_t, in_=xb.ap()[128 * tt:128 * tt + 128, :])
                    for jb in range(4):
                        tp = ps.tile([128, 4, 128], F32, tag="scr", bufs=1)
                        for jj in range(4):
                            nc.tensor.transpose(
                                tp[:, jj, :], x_t[:, 128 * (4 * jb + jj):128 * (4 * jb + jj) + 128], ident)
                        nc.vector.tensor_copy(
                            xTp[:, 4 * jb:4 * jb + 4, 128 * tl:128 * tl + 128], tp)
                # projection for this pair, col-packed (2 concurrent col groups)
                hp = ps.tile([128, 128], F32, tag="scr", bufs=1)
                for k in range(16):
                    for g2 in range(2):
                        nc.tensor.matmul(
                            out=hp[64 * g2:64 * g2 + 64, :],
                            lhsT=t_W[:, k, :],
                            rhs=xTp[:, k, 128 * g2:128 * g2 + 128],
                            start=(k == 0), stop=(k == 15),
                            tile_position=(0, 64 * g2),
                        )
                for g2 in range(2):
                    s0 = 128 * (4 * g + 2 * pair + g2)
                    nc.vector.scalar_tensor_tensor(
                        out=h_buf[0:64, s0:s0 + 128], in0=hp[64 * g2:64 * g2 + 64, :],
                        scalar=1.0, in1=t_bp[:, 0:1].broadcast_to([64, 128]),
                        op0=OP.mult, op1=OP.add)
                    nc.vector.tensor_copy(r_h[0:64, s0:s0 + 128], h_buf[0:64, s0:s0 + 128])

            # ---- pool phases for this group's 4 tiles ----
            for tl in range(4):
                tt = 4 * g + tl
                Zs = sb2.tile([128, 4], F32, tag="Zs", bufs=3)
                Zall = sb2.tile([128, 3], F32, tag="Zall", bufs=3)
                Es = {}
                for (ph, c0, nch) in PHASES:
                    lp = ps.tile([128, 1024], F32, tag="lg", bufs=2)
                    for ci in range(nch):
                        c = c0 + ci
                        nc.tensor.matmul(
                            out=lp[:, 512 * ci:512 * ci + 512],
                            lhsT=r_h[:, 128 * tt:128 * tt + 128],
                            rhs=r_emb[:, 512 * c:512 * c + 512],
                            start=True, stop=True)
                    E = sb2.tile([128, 1024], BF16, tag="E", bufs=6)
                    acc = {"F1": Zs[:, 0:1], "F2": Zs[:, 1:2], "R": Zall[:, 1:2],
                           "V1": Zs[:, 2:3], "V2": Zs[:, 3:4]}[ph]
                    nc.scalar.activation(E[:, 0:512 * nch], lp[:, 0:512 * nch],
                                         AF.Exp, bias=0.0, scale=1.0, accum_out=acc)
                    Es[ph] = E
                # Z finalize + weights
                nc.vector.tensor_add(out=Zall[:, 0:1], in0=Zs[:, 0:1], in1=Zs[:, 1:2])
                nc.vector.tensor_add(out=Zall[:, 2:3], in0=Zs[:, 2:3], in1=Zs[:, 3:4])
                rZ3 = sb2.tile([128, 3], F32, tag="rZ3", bufs=3)
                nc.vector.reciprocal(rZ3, Zall)
                a3 = sb2.tile([128, 3], F32, tag="a3", bufs=3)
                nc.vector.tensor_scalar_mul(a3, rZ3, t_imp[:, tt:tt + 1])
                nc.vector.tensor_copy(a48[:, 3 * tt:3 * tt + 3], a3)
                ac = sb2.tile([128, 3, 2], BF16, tag="ac", bufs=3)
                nc.vector.tensor_copy(ac[:, :, 0], a3)
                nc.vector.tensor_copy(ac[:, :, 1], rZ3)
                # dense matmuls (col-packed accumulation into persistent banks)
                for c in range(NCHUNK):
                    pool = 0 if c < 4 else (1 if c < 6 else 2)
                    ph, off = [("F1", 0), ("F1", 1), ("F2", 0), ("F2", 1), ("R", 0),
                               ("R", 1), ("V1", 0), ("V1", 1), ("V2", 0)][c]
                    nc.tensor.matmul(
                        out=pd[c // 4][32 * (c % 4):32 * (c % 4) + 2, :],
                        lhsT=ac[:, pool, :],
                        rhs=Es[ph][:, 512 * off:512 * off + 512],
                        start=(tt == 0), stop=(tt == NT - 1),
                        tile_position=(0, 32 * (c % 4)),
                    )

        # ---- evacuate outputs ----
        od = sb.tile([128, 3 * 512], F32, tag="od")
        for i in range(3):
            nc.vector.tensor_copy(od[:, 512 * i:512 * i + 512], pd[i])
        nc.sync.dma_start(out=dpacked.ap(), in_=od)
        nc.sync.dma_start(out=h_out.ap(), in_=h_buf)
        nc.sync.dma_start(out=a48_out.ap(), in_=a48)

    nc.compile()
    return nc


def _build_call2():
    nc = bacc.Bacc("TRN2", target_bir_lowering=False, debug=False, num_devices=8)
    NC3 = 3 * NCAND

    h_in = nc.dram_tensor("h_in", [65, S], F32, kind="ExternalInput")
    ecand = nc.dram_tensor("ecand", [65, NC3], F32, kind="ExternalInput")
    a48 = nc.dram_tensor("a48", [128, 3 * NT], F32, kind="ExternalInput")
    dcand = nc.dram_tensor("dcand", [1, NC3], F32, kind="ExternalOutput")

    with tile.TileContext(nc) as tc, ExitStack() as ctx:
        sb = ctx.enter_context(tc.tile_pool(name="sb", bufs=1))
        sb2 = ctx.enter_context(tc.tile_pool(name="sb2", bufs=1))
        ps = ctx.enter_context(tc.tile_pool(name="ps", bufs=1, space="PSUM"))

        t_h = sb.tile([65, S], F32, tag="h")
        nc.sync.dma_start(out=t_h, in_=h_in.ap())
        t_e = sb.tile([65, NC3], F32, tag="e")
        nc.sync.dma_start(out=t_e, in_=ecand.ap())
        t_a = sb.tile([128, 3 * NT], F32, tag="a")
        nc.sync.dma_start(out=t_a, in_=a48.ap())

        pdc = []
        for p in range(3):
            pdc_p = ps.tile([1, NCAND], F32, tag=f"pd{p}", name=f"pdc{p}")
            pdc.append(pdc_p)
        for tt in range(NT):
            lp = ps.tile([128, NC3], F32, tag="lg", bufs=2)
            nc.tensor.matmul(out=lp, lhsT=t_h[:, 128 * tt:128 * tt + 128],
                             rhs=t_e, start=True, stop=True)
            E = sb2.tile([128, NC3], F32, tag="E", bufs=3)
            nc.scalar.activation(E, lp, AF.Exp, bias=0.0, scale=1.0)
            for p in range(3):
                nc.tensor.matmul(
                    out=pdc[p],
                    lhsT=t_a[:, 3 * tt + p:3 * tt + p + 1],
                    rhs=E[:, NCAND * p:NCAND * p + NCAND],
                    start=(tt == 0), stop=(tt == NT - 1))
        oc = sb.tile([1, NC3], F32, tag="oc")
        for p in range(3):
            nc.vector.tensor_copy(oc[:, NCAND * p:NCAND * p + NCAND], pdc[p])
        nc.sync.dma_start(out=dcand.ap(), in_=oc)

    nc.compile()
    return nc


def _get(name, builder):
    if name not in _CACHE:
        _CACHE[name] = builder()
    return _CACHE[name]


def kernel(x, importance, W_proj, b_proj, neuron_emb, usage_f, usage_r, usage_v):
    x = np.ascontiguousarray(np.asarray(x, np.float32))
    importance = np.asarray(importance, np.float32)
    W_proj = np.ascontiguousarray(np.asarray(W_proj, np.float32))
    b_proj = np.asarray(b_proj, np.float32)
    neuron_emb = np.asarray(neuron_emb, np.float32)
    usage_f = np.asarray(usage_f, np.float32)
    usage_r = np.asarray(usage_r, np.float32)
    usage_v = np.asarray(usage_v, np.float32)

    cores = list(range(B))

    # host prep (replicated small weights)
    emb = neuron_emb[:NTOT]
    norm = np.sqrt((emb * emb).sum(axis=1, dtype=np.float32)).astype(np.float32)
    embn = emb / (norm + np.float32(1e-12))[:, None]
    exc = np.concatenate([1.0 - usage_f, 1.0 - usage_r, 1.0 - usage_v]).astype(np.float32)
    np.clip(exc, 0.0, 1.0, out=exc)
    embT_ext = np.concatenate([embn.T, exc[None, :]], axis=0)
    embT_ext = np.ascontiguousarray(embT_ext, np.float32)
    bp = np.ascontiguousarray(b_proj.reshape(DS, 1))

    nc1 = _get("c1", _build_call1)
    in_maps = []
    for b in range(B):
        in_maps.append({
            "xb": x[b],
            "imp2d": np.ascontiguousarray(importance[b].reshape(NT, 128).T),
            "Wt": W_proj, "bproj": bp, "embT": embT_ext,
        })
    import time as _time
    _t0 = _time.time()
    res1 = run_bass_kernel_spmd(nc1, in_maps, cores).results
    LAST_TIMING["call1_wall_s"] = _time.time() - _t0

    # unpack approx dense + colsum; pick candidates
    dense_a = np.empty((B, NTOT), np.float32)
    colsum = np.empty((B, NTOT), np.float32)
    for b in range(B):
        dpk = res1[b]["dpacked"]
        for c in range(NCHUNK):
            g = 32 * (c % 4)
            dense_a[b, 512 * c:512 * c + 512] = dpk[g, 512 * (c // 4):512 * (c // 4) + 512]
            colsum[b, 512 * c:512 * c + 512] = dpk[g + 1, 512 * (c // 4):512 * (c // 4) + 512]

    pools = [(0, NF, KF), (NF, NR, KR), (NF + NR, NV, KV)]
    cand_idx = np.empty((B, 3, NCAND), np.int64)
    in_maps2 = []
    for b in range(B):
        ec = np.empty((65, 3 * NCAND), np.float32)
        for p, (o, n, k) in enumerate(pools):
            idx = np.argpartition(dense_a[b, o:o + n], n - NCAND)[-NCAND:]
            cand_idx[b, p] = idx
            ec[:, NCAND * p:NCAND * p + NCAND] = embT_ext[:, o + idx]
        in_maps2.append({"h_in": res1[b]["h_out"],
                         "ecand": np.ascontiguousarray(ec),
                         "a48": res1[b]["a48_out"]})

    nc2 = _get("c2", _build_call2)
    _t0 = _time.time()
    res2 = run_bass_kernel_spmd(nc2, in_maps2, cores).results
    LAST_TIMING["call2_wall_s"] = _time.time() - _t0

    # final top-k + renormalize + scatter
    outs = [np.zeros((B, NF), np.float32), np.zeros((B, NR), np.float32),
            np.zeros((B, NV), np.float32)]
    for b in range(B):
        dc = res2[b]["dcand"][0]
        for p, (o, n, k) in enumerate(pools):
            vals = dc[NCAND * p:NCAND * p + NCAND]
            order = np.argsort(vals)[::-1][:k]
            top_idx = cand_idx[b, p][order]
            top_vals = vals[order]
            w = top_vals / (top_vals.sum(dtype=np.float32) + np.float32(1e-8))
            outs[p][b, top_idx] = w

    # aux loss from colsums (the cross-core all-reduce)
    u = colsum.sum(axis=0, dtype=np.float32) / np.float32(B * S)
    aux = np.float32(0.0)
    for p, (o, n, k) in enumerate(pools):
        up = u[o:o + n]
        bal = ((up - np.float32(1.0 / n)) ** 2).sum(dtype=np.float32) * np.float32(n)
        aux += bal * (2.0 if p == 1 else 1.0)

    feature_weights, rel_q, value_weights = outs
    return (feature_weights, rel_q, rel_q.copy(), value_weights, np.float32(aux))
